# revision 12
# baseline (speedup 1.0000x reference)
"""GraphTransformerLayer on 8 TRN2 NeuronCores (Bass/Tile).

Sharding: query/node dim N=2048 split into 8 shards of 256 rows.
Each core computes full K/V (replicated) + attention/FFN for its shard.
Edge bias is scattered on host into a dense per-core (H, N_keys, 256)
slab; softmax is unnormalized-exp (scores are bounded ~|1|) with the
denominator computed as an extra all-ones column of V.
"""

import sys

sys.path.insert(0, "/opt/trn_rl_repo")

import numpy as np

import concourse.bacc as bacc
import concourse.mybir as mybir
import concourse.tile as tile
from concourse.bass_utils import run_bass_kernel_spmd

N_CORES = 8
N = 2048
D = 256
H = 8
DK = 32
QS = N // N_CORES  # 256 query rows per core
H2 = 512
EPS = 1e-5

F32 = mybir.dt.float32
FR = mybir.dt.float32r
BF = mybir.dt.bfloat16





def build_kernel(use_fr=True):
    MT = F32
    AT = BF if use_fr else F32
    nc = bacc.Bacc("TRN2", target_bir_lowering=False, debug=False,
                   num_devices=N_CORES)

    d_hT = nc.dram_tensor("hT", [D, N], F32, kind="ExternalInput")
    d_hTs = nc.dram_tensor("hTs", [D, QS], F32, kind="ExternalInput")
    d_hres = nc.dram_tensor("hres", [QS, D], F32, kind="ExternalInput")
    d_biasT = nc.dram_tensor("biasT", [H, N, QS], F32, kind="ExternalInput")
    d_wq = nc.dram_tensor("wq", [D, D], F32, kind="ExternalInput")
    d_wk = nc.dram_tensor("wk", [D, D], F32, kind="ExternalInput")
    d_wv = nc.dram_tensor("wv", [D, 272], F32, kind="ExternalInput")
    d_bq = nc.dram_tensor("bq", [D, 1], F32, kind="ExternalInput")
    d_bk = nc.dram_tensor("bk", [D, 1], F32, kind="ExternalInput")
    d_bv = nc.dram_tensor("bv", [1, 272], F32, kind="ExternalInput")
    d_wo = nc.dram_tensor("wo", [D, D], F32, kind="ExternalInput")
    d_bo = nc.dram_tensor("bo", [1, D], F32, kind="ExternalInput")
    d_g1 = nc.dram_tensor("g1", [128, D], F32, kind="ExternalInput")
    d_be1 = nc.dram_tensor("be1", [128, D], F32, kind="ExternalInput")
    d_g2 = nc.dram_tensor("g2", [128, D], F32, kind="ExternalInput")
    d_be2 = nc.dram_tensor("be2", [128, D], F32, kind="ExternalInput")
    d_w1 = nc.dram_tensor("w1", [D, H2], F32, kind="ExternalInput")
    d_b1 = nc.dram_tensor("b1", [H2, 1], F32, kind="ExternalInput")
    d_w2 = nc.dram_tensor("w2", [H2, D], F32, kind="ExternalInput")
    d_b2 = nc.dram_tensor("b2", [D, 1], F32, kind="ExternalInput")
    d_id = nc.dram_tensor("ident", [128, 128], F32, kind="ExternalInput")
    d_out = nc.dram_tensor("out", [QS, D], F32, kind="ExternalOutput")

    with tile.TileContext(nc) as tc:
        import contextlib

        with contextlib.ExitStack() as ctx:
            wpool = ctx.enter_context(tc.tile_pool(name="weights", bufs=1))
            big = ctx.enter_context(tc.tile_pool(name="big", bufs=1))
            ptp = ctx.enter_context(tc.tile_pool(name="pt", bufs=2))
            bias_p = ctx.enter_context(tc.tile_pool(name="bias", bufs=6))
            sm = ctx.enter_context(tc.tile_pool(name="small", bufs=2))
            smk = ctx.enter_context(tc.tile_pool(name="smallk", bufs=1))
            ps_a = ctx.enter_context(
                tc.tile_pool(name="psA", bufs=2, space="PSUM"))
            ps_st = ctx.enter_context(
                tc.tile_pool(name="psST", bufs=3, space="PSUM"))
            ps_o = ctx.enter_context(
                tc.tile_pool(name="psO", bufs=2, space="PSUM"))

            # ---------- load weights / inputs ----------
            def load(pool, dram, shape, row0=0, col0=0, name=None, dt=F32):
                t = pool.tile(shape, dt, name=name or f"{dram.name}_sb_{row0}_{col0}")
                nc.sync.dma_start(
                    t[:], dram.ap()[row0:row0 + shape[0],
                                    col0:col0 + shape[1]])
                return t

            hT = [load(big, d_hT, [128, N], 128 * i) for i in range(2)]
            hTs = [load(big, d_hTs, [128, QS], 128 * i) for i in range(2)]
            hres = [load(big, d_hres, [128, D], 128 * i) for i in range(2)]
            wq = [load(wpool, d_wq, [128, D], 128 * i) for i in range(2)]
            wk = [load(wpool, d_wk, [128, D], 128 * i) for i in range(2)]
            wv = [load(wpool, d_wv, [128, 272], 128 * i) for i in range(2)]
            wo = [load(wpool, d_wo, [128, D], 128 * i) for i in range(2)]
            w1 = [load(wpool, d_w1, [128, H2], 128 * i) for i in range(2)]
            w2 = [load(wpool, d_w2, [128, D], 128 * i) for i in range(4)]
            bq = [load(wpool, d_bq, [128, 1], 128 * i) for i in range(2)]
            bk = [load(wpool, d_bk, [128, 1], 128 * i) for i in range(2)]
            b1 = [load(wpool, d_b1, [128, 1], 128 * i) for i in range(4)]
            b2 = [load(wpool, d_b2, [128, 1], 128 * i) for i in range(2)]
            bv = load(wpool, d_bv, [1, 272])
            bo = load(wpool, d_bo, [1, D])
            g1t = load(wpool, d_g1, [128, D])
            be1t = load(wpool, d_be1, [128, D])
            g2t = load(wpool, d_g2, [128, D])
            be2t = load(wpool, d_be2, [128, D])
            ident = load(wpool, d_id, [128, 128])
            ones = wpool.tile([1, 128], F32, name="ones")
            nc.vector.memset(ones[:], 1.0)
            zcol = wpool.tile([128, 1], F32, name="zcol")
            nc.vector.memset(zcol[:], 0.0)
            epscol = wpool.tile([128, 1], F32, name="epscol")
            nc.vector.memset(epscol[:], EPS)

            # ---------- projections ----------
            # QT[o, q] (2 tiles of 128): lhsT = wq chunk, rhs = hTs chunk
            QT = []
            for oc in range(2):
                ps = ps_a.tile([128, QS], F32, tag="psa", name="psq")
                for ic in range(2):
                    nc.tensor.matmul(
                        ps[:], (wq[ic][:, 128 * oc:128 * oc + 128]),
                        hTs[ic][:],
                        start=(ic == 0), stop=(ic == 1))
                t = big.tile([128, QS], AT, tag=f"QT{oc}", name=f"QT{oc}")
                nc.scalar.activation(t[:], ps[:],
                                     mybir.ActivationFunctionType.Identity,
                                     bias=bq[oc][:])
                QT.append(t)

            KT = [big.tile([128, N], AT, tag=f"KT{oc}", name=f"KT{oc}") for oc in range(2)]
            for oc in range(2):
                for fc in range(4):
                    ps = ps_a.tile([128, 512], F32, tag="psa", name="psk")
                    for ic in range(2):
                        nc.tensor.matmul(
                            ps[:],
                            (wk[ic][:, 128 * oc:128 * oc + 128]),
                            (hT[ic][:, 512 * fc:512 * fc + 512]),
                            start=(ic == 0), stop=(ic == 1))
                    nc.scalar.activation(
                        KT[oc][:, 512 * fc:512 * fc + 512], ps[:],
                        mybir.ActivationFunctionType.Identity, bias=bk[oc][:])

            # V natural (node, feat) augmented with per-head ones column:
            # v_sb[:, 264*c + 33*h + j]
            v_sb = big.tile([128, 16 * 272], AT, name="v_sb")
            for cchunk in range(16):
                ps = ps_a.tile([128, 272], F32, tag="psa", name="psv")
                for ic in range(2):
                    nc.tensor.matmul(
                        ps[:],
                        (hT[ic][:, 128 * cchunk:128 * cchunk + 128]),
                        wv[ic][:],
                        start=(ic == 0), stop=False)
                nc.tensor.matmul(ps[:], ones[:],
                                 bv[:],
                                 start=False, stop=True)
                nc.vector.tensor_copy(
                    v_sb[:, 272 * cchunk:272 * cchunk + 272], ps[:])

            # ---------- attention ----------
            o_nat = [big.tile([128, D], F32, tag=f"onat{qt}", name=f"onat{qt}")
                     for qt in range(2)]
            for h in range(8):
                tl, bp = h // 4, 32 * (h % 4)
                pt = ptp.tile([128, 16 * QS], AT, tag="pt", name="pt")
                for c in range(16):
                    ps = ps_st.tile([128, QS], F32, tag="pst", name="st_ps")
                    nc.tensor.matmul(
                        ps[:],
                        (KT[tl][bp:bp + 32, 128 * c:128 * c + 128]),
                        (QT[tl][bp:bp + 32, :]),
                        start=True, stop=True, tile_position=(bp, 0))
                    bt = bias_p.tile([128, QS], F32, tag="bias", name="bias_t")
                    nc.sync.dma_start(
                        bt[:], d_biasT.ap()[h, 128 * c:128 * c + 128, :])
                    nc.vector.tensor_add(
                        pt[:, QS * c:QS * c + QS], ps[:], bt[:])
                nc.scalar.activation(pt[:], pt[:],
                                     mybir.ActivationFunctionType.Exp,
                                     bias=zcol[:])
                for qt in range(2):
                    ops = ps_o.tile([128, 34], F32, tag="o", name="o_ps")
                    for c in range(16):
                        nc.tensor.matmul(
                            ops[:],
                            (
                                pt[:, QS * c + 128 * qt:QS * c + 128 * qt + 128]),
                            (
                                v_sb[:, 272 * c + 34 * h:272 * c + 34 * h + 34]),
                            start=(c == 0), stop=(c == 15))
                    rden = sm.tile([128, 1], F32, tag="rden", name="rden")
                    nc.vector.reciprocal(rden[:], ops[:, 32:33])
                    nc.vector.tensor_scalar_mul(
                        o_nat[qt][:, 32 * h:32 * h + 32], ops[:, 0:32],
                        rden[:])

            # ---------- output projection + residual + LN ----------
            OT = [big.tile([128, D], F32, tag=f"OT{fc}", name=f"OT{fc}") for fc in range(2)]
            for qt in range(2):
                for fc in range(2):
                    tps = ps_a.tile([128, 128], F32, tag="psa", name="tr_ps")
                    nc.tensor.transpose(
                        tps[:], o_nat[qt][:, 128 * fc:128 * fc + 128],
                        ident[:])
                    nc.vector.tensor_copy(
                        OT[fc][:, 128 * qt:128 * qt + 128], tps[:])

            def layer_norm(src_tiles, gamma, beta, out_tag):
                outs = []
                for qt in range(2):
                    x = src_tiles[qt]
                    ssum = sm.tile([128, 1], F32, tag="lnsum")
                    nc.vector.reduce_sum(ssum[:], x[:],
                                         axis=mybir.AxisListType.X)
                    negmean = sm.tile([128, 1], F32, tag="lnneg")
                    nc.scalar.mul(negmean[:], ssum[:], -1.0 / D)
                    xc = sm.tile([128, D], F32, tag="lnxc")
                    nc.scalar.activation(
                        xc[:], x[:], mybir.ActivationFunctionType.Identity,
                        bias=negmean[:])
                    scr = sm.tile([128, D], F32, tag="lnscr")
                    vs = sm.tile([128, 1], F32, tag="lnvs")
                    nc.scalar.activation(
                        scr[:], xc[:], mybir.ActivationFunctionType.Square,
                        bias=zcol[:], accum_out=vs[:])
                    st = sm.tile([128, 1], F32, tag="lnstd")
                    nc.scalar.activation(
                        st[:], vs[:], mybir.ActivationFunctionType.Sqrt,
                        bias=epscol[:], scale=1.0 / D)
                    r0 = sm.tile([128, 1], F32, tag="lnr0")
                    nc.vector.reciprocal(r0[:], st[:])
                    # one Newton step for rsqrt accuracy:
                    # r1 = r0*(1.5 - 0.5*v*r0^2), v = vs/D + eps
                    vv = sm.tile([128, 1], F32, tag="lnvv")
                    nc.vector.tensor_scalar(
                        vv[:], vs[:], 1.0 / D, EPS,
                        op0=mybir.AluOpType.mult, op1=mybir.AluOpType.add)
                    rr = sm.tile([128, 1], F32, tag="lnrr")
                    nc.vector.tensor_mul(rr[:], r0[:], r0[:])
                    va = sm.tile([128, 1], F32, tag="lnva")
                    nc.vector.tensor_mul(va[:], vv[:], rr[:])
                    cc = sm.tile([128, 1], F32, tag="lncc")
                    nc.vector.tensor_scalar(
                        cc[:], va[:], -0.5, 1.5,
                        op0=mybir.AluOpType.mult, op1=mybir.AluOpType.add)
                    r1 = sm.tile([128, 1], F32, tag="lnr1")
                    nc.vector.tensor_mul(r1[:], r0[:], cc[:])
                    yp = sm.tile([128, D], F32, tag="lnyp")
                    nc.vector.tensor_scalar_mul(yp[:], xc[:], r1[:])
                    yg = sm.tile([128, D], F32, tag=f"{out_tag}{qt}")
                    nc.vector.tensor_mul(yg[:], yp[:], gamma[:])
                    nc.vector.tensor_add(yg[:], yg[:], beta[:])
                    outs.append(yg)
                return outs

            xin = []
            for qt in range(2):
                aps = ps_a.tile([128, D], F32, tag="psa", name="att_ps")
                for ic in range(2):
                    nc.tensor.matmul(
                        aps[:],
                        (OT[ic][:, 128 * qt:128 * qt + 128]),
                        wo[ic][:],
                        start=(ic == 0), stop=False)
                nc.tensor.matmul(aps[:], ones[:],
                                 bo[:],
                                 start=False, stop=True)
                x = smk.tile([128, D], F32, tag=f"xin{qt}", name=f"xin{qt}")
                nc.vector.tensor_add(x[:], aps[:], hres[qt][:])
                xin.append(x)

            h1 = layer_norm(xin, g1t, be1t, "h1")
            # keep h1 tiles alive in smk pool (bufs=1, unique tags)
            h1k = []
            for qt in range(2):
                t = smk.tile([128, D], F32, tag=f"h1k{qt}", name=f"h1k{qt}")
                nc.vector.tensor_copy(t[:], h1[qt][:])
                h1k.append(t)
            fln = layer_norm(h1k, g2t, be2t, "fln")

            # ---------- FFN ----------
            fT = [smk.tile([128, D], F32, tag=f"fT{ic}", name=f"fT{ic}") for ic in range(2)]
            for qt in range(2):
                for fc in range(2):
                    tps = ps_a.tile([128, 128], F32, tag="psa", name="tr2_ps")
                    nc.tensor.transpose(
                        tps[:], fln[qt][:, 128 * fc:128 * fc + 128], ident[:])
                    nc.vector.tensor_copy(
                        fT[fc][:, 128 * qt:128 * qt + 128], tps[:])

            g1T = [smk.tile([128, QS], F32, tag=f"g1T{oc}", name=f"g1T{oc}") for oc in range(4)]
            for oc in range(4):
                ps = ps_st.tile([128, QS], F32, tag="pst", name="ffn1_ps")
                for ic in range(2):
                    nc.tensor.matmul(
                        ps[:],
                        (w1[ic][:, 128 * oc:128 * oc + 128]),
                        fT[ic][:],
                        start=(ic == 0), stop=(ic == 1))
                nc.scalar.activation(
                    g1T[oc][:], ps[:], mybir.ActivationFunctionType.Gelu,
                    bias=b1[oc][:])

            y2T = [smk.tile([128, QS], F32, tag=f"y2T{oc}", name=f"y2T{oc}") for oc in range(2)]
            for oc in range(2):
                ps = ps_st.tile([128, QS], F32, tag="pst", name="ffn2_ps")
                for ic in range(4):
                    nc.tensor.matmul(
                        ps[:],
                        (w2[ic][:, 128 * oc:128 * oc + 128]),
                        g1T[ic][:],
                        start=(ic == 0), stop=(ic == 3))
                nc.scalar.activation(
                    y2T[oc][:], ps[:], mybir.ActivationFunctionType.Identity,
                    bias=b2[oc][:])

            out_sb = [smk.tile([128, D], F32, tag=f"out{qt}", name=f"outsb{qt}")
                      for qt in range(2)]
            for qt in range(2):
                for fc in range(2):
                    tps = ps_a.tile([128, 128], F32, tag="psa", name="tr3_ps")
                    nc.tensor.transpose(
                        tps[:], y2T[fc][:, 128 * qt:128 * qt + 128], ident[:])
                    nc.vector.tensor_add(
                        out_sb[qt][:, 128 * fc:128 * fc + 128],
                        h1k[qt][:, 128 * fc:128 * fc + 128], tps[:])
                nc.sync.dma_start(d_out.ap()[128 * qt:128 * qt + 128, :],
                                  out_sb[qt][:])

    nc.compile()
    return nc


_CACHE = {}
USE_FR = True


def _get_nc(use_fr=True):
    if use_fr not in _CACHE:
        _CACHE[use_fr] = build_kernel(use_fr)
    return _CACHE[use_fr]


def kernel(**inputs):
    h = np.asarray(inputs["h"], np.float32)
    edge_attr = np.asarray(inputs["edge_attr"], np.float32)
    edge_index = np.asarray(inputs["edge_index"])
    Wq, bq = np.asarray(inputs["Wq"], np.float32), np.asarray(inputs["bq"], np.float32)
    Wk, bk = np.asarray(inputs["Wk"], np.float32), np.asarray(inputs["bk"], np.float32)
    Wv, bv = np.asarray(inputs["Wv"], np.float32), np.asarray(inputs["bv"], np.float32)
    Wo, bo = np.asarray(inputs["Wo"], np.float32), np.asarray(inputs["bo"], np.float32)
    We, be = np.asarray(inputs["We"], np.float32), np.asarray(inputs["be"], np.float32)
    ln1_g, ln1_b = np.asarray(inputs["ln1_g"], np.float32), np.asarray(inputs["ln1_b"], np.float32)
    fln_g, fln_b = np.asarray(inputs["fln_g"], np.float32), np.asarray(inputs["fln_b"], np.float32)
    W1, b1 = np.asarray(inputs["W1"], np.float32), np.asarray(inputs["b1"], np.float32)
    W2, b2 = np.asarray(inputs["W2"], np.float32), np.asarray(inputs["b2"], np.float32)

    scale = 1.0 / np.sqrt(np.float32(DK))
    eb = edge_attr @ We + be  # (E, H)

    hT = np.ascontiguousarray(h.T)  # (D, N)
    wv_aug = np.zeros((D, 272), np.float32)
    bv_aug = np.zeros((1, 272), np.float32)
    for hh in range(H):
        wv_aug[:, 34 * hh:34 * hh + 32] = Wv[:, 32 * hh:32 * hh + 32]
        bv_aug[0, 34 * hh:34 * hh + 32] = bv[32 * hh:32 * hh + 32]
        bv_aug[0, 34 * hh + 32] = 1.0

    common = {
        "hT": hT,
        "wq": (Wq * scale).astype(np.float32),
        "wk": Wk, "wv": wv_aug,
        "bq": (bq * scale).reshape(D, 1).astype(np.float32),
        "bk": bk.reshape(D, 1), "bv": bv_aug,
        "wo": Wo, "bo": bo.reshape(1, D),
        "g1": np.tile(ln1_g, (128, 1)), "be1": np.tile(ln1_b, (128, 1)),
        "g2": np.tile(fln_g, (128, 1)), "be2": np.tile(fln_b, (128, 1)),
        "w1": W1, "b1": b1.reshape(H2, 1),
        "w2": W2, "b2": b2.reshape(D, 1),
        "ident": np.eye(128, dtype=np.float32),
    }

    src = edge_index[0].astype(np.int64)
    dst = edge_index[1].astype(np.int64)
    in_maps = []
    for c in range(N_CORES):
        r0 = c * QS
        m = dict(common)
        m["hTs"] = np.ascontiguousarray(hT[:, r0:r0 + QS])
        m["hres"] = np.ascontiguousarray(h[r0:r0 + QS])
        biasT = np.zeros((H, N, QS), np.float32)
        sel = (src >= r0) & (src < r0 + QS)
        biasT[:, dst[sel], src[sel] - r0] = eb[sel].T
        m["biasT"] = biasT
        in_maps.append(m)

    nc = _get_nc(use_fr=USE_FR)
    res = run_bass_kernel_spmd(nc, in_maps, core_ids=list(range(N_CORES)))
    out = np.concatenate([res.results[c]["out"] for c in range(N_CORES)],
                         axis=0)
    return out.astype(np.float32)
